# revision 1
# baseline (speedup 1.0000x reference)
"""DiffPathRenderer Trainium2 kernel, v3.

Layout A (partition = pixel row, free = pixel col, one [128,128] image per
segment, running min across segments) with the affine t1 = B*i + (A*j + C)
generated on the TensorEngine: one K=2 matmul per segment,
lhsT = [[i],[1]] (const), rhs = [[B]*128, A*j+C] (host-precomputed).

Per segment:
  t1  = matmul -> PSUM                       PE
  t   = clip(t1, 0, 1)                       DVE (PSUM read)
  m   = -wvx*t + X                           DVE stt
  sx  = (m - vx)^2                           ACT Square (bias col)
  sy  = (-wvy*t + (i-vy))^2                  ACT Square (scale+bias cols)
  w   = sx + sy                              GPSIMD add
  dmin= min(dmin, w)                         alternating DVE / GPSIMD

Finals batched across all 32 strokes (single Sqrt + single Sigmoid table
load), one rearranged DMA out.
"""

import numpy as np

import concourse.bacc as bacc
import concourse.mybir as mybir
import concourse.tile as tile
from concourse.bass_utils import run_bass_kernel_spmd

F32 = mybir.dt.float32
N_CORES = 8
B_TOTAL = 256
B_CORE = B_TOTAL // N_CORES
NSEG = 16
G_CORE = B_CORE * NSEG           # 512
P = 128

# allin columns: 4 coef types x 512 segs | X (128) | invr2, bias7
C_NWVX, C_NVX, C_NWVY, C_IVY = range(4)
X_OFF = 4 * G_CORE
IR_OFF = X_OFF + P
W_ALL = IR_OFF + 2
LH_OFF = IR_OFF + 2              # lhsT const [2,128] on partitions 0-1
W_ALL2 = LH_OFF + P

_cached = {}


def _build_bass():
    nc = bacc.Bacc(None)
    allin = nc.declare_dram_parameter("allin", [P, W_ALL2], F32, isOutput=False)
    rhin = nc.declare_dram_parameter("rhin", [2 * B_CORE, NSEG * P], F32,
                                     isOutput=False)
    out = nc.declare_dram_parameter("out", [B_CORE, P, P], F32, isOutput=True)

    AL = mybir.AluOpType
    AF = mybir.ActivationFunctionType

    with tile.TileContext(nc) as tc:
        with tc.tile_pool(name="main", bufs=1) as cpool, \
             tc.tile_pool(name="work", bufs=4) as wp, \
             tc.tile_pool(name="psum", bufs=4, space="PSUM") as pp:
            atile = cpool.tile([P, W_ALL2], F32, name="atile")
            nc.gpsimd.dma_start(out=atile[:], in_=allin[:])

            def col(cid, g):
                j = cid * G_CORE + g
                return atile[:, j:j + 1]

            xt = atile[:, X_OFF:X_OFF + P]
            irt0 = atile[:, IR_OFF:IR_OFF + 1]
            irt1 = atile[:, IR_OFF + 1:IR_OFF + 2]

            LHS = atile[0:2, LH_OFF:LH_OFF + P]

            dma_ = cpool.tile([P, B_CORE * P], F32, name="dma_")

            for k in range(B_CORE):
                dmin = dma_[:, k * P:(k + 1) * P]
                rh = wp.tile([2, NSEG * P], F32, tag="rh", bufs=3, name="rh")
                nc.sync.dma_start(out=rh[:], in_=rhin[2 * k:2 * k + 2, :])
                for s4 in range(NSEG // 4):
                    # one matmul + one batched clip for 4 segments
                    pt = pp.tile([P, 4 * P], F32, tag="pt", name="pt")
                    nc.tensor.matmul(pt[:], LHS,
                                     rh[:, s4 * 4 * P:(s4 + 1) * 4 * P],
                                     start=True, stop=True)
                    t4 = wp.tile([P, 4 * P], F32, tag="t4", name="t4")
                    nc.vector.tensor_scalar(t4[:], pt[:], 0.0, 1.0, AL.max, AL.min)
                    for ss in range(4):
                        s = s4 * 4 + ss
                        g = k * NSEG + s
                        t = t4[:, ss * P:(ss + 1) * P]
                        m = wp.tile([P, P], F32, tag="m", name="m")
                        nc.vector.scalar_tensor_tensor(m[:], t, col(C_NWVX, g),
                                                       xt, AL.mult, AL.add)
                        sx = wp.tile([P, P], F32, tag="sx", name="sx")
                        nc.scalar.activation(sx[:], m[:], AF.Square,
                                             bias=col(C_NVX, g))
                        sy = wp.tile([P, P], F32, tag="sy", name="sy")
                        nc.scalar.activation(sy[:], t, AF.Square,
                                             bias=col(C_IVY, g),
                                             scale=col(C_NWVY, g))
                        if s == 0:
                            nc.gpsimd.tensor_tensor(dmin, sx[:], sy[:], AL.add)
                        else:
                            w = wp.tile([P, P], F32, tag="w", name="w")
                            nc.gpsimd.tensor_tensor(w[:], sx[:], sy[:], AL.add)
                            nc.vector.tensor_tensor(dmin, dmin, w[:], AL.min)

            # finals
            FB = B_CORE * P
            nc.vector.tensor_scalar_max(dma_[:], dma_[:], 0.0)
            qd = cpool.tile([P, FB], F32, name="qd")
            nc.scalar.activation(qd[:], dma_[:], AF.Sqrt, scale=irt0)
            nc.vector.tensor_scalar_min(qd[:], qd[:], 1.0)
            nc.scalar.activation(dma_[:], qd[:], AF.Sigmoid, scale=-70.0, bias=irt1)
            ov = out[:].rearrange("k i j -> i k j")
            sv = dma_[:].rearrange("i (k j) -> i k j", k=B_CORE)
            nc.sync.dma_start(out=ov, in_=sv)
    nc.finalize()
    return nc


def _host_coefs(traj, thickness):
    traj = np.asarray(traj, dtype=np.float32)
    T = traj * np.float32(128.0)
    v = T[:, :-1]
    w = T[:, 1:]
    wv = w - v
    d = np.sqrt(wv[..., 0] ** 2 + wv[..., 1] ** 2)
    e2 = d * d + np.float32(1e-5)
    inv = np.float32(1.0) / e2
    A = wv[..., 0] * inv                           # (256,16)
    Bc = wv[..., 1] * inv
    C = -(v[..., 0] * wv[..., 0] + v[..., 1] * wv[..., 1]) * inv

    ii = np.arange(P, dtype=np.float32)
    ones = np.ones(P, dtype=np.float32)

    cNWvx = (-wv[..., 0])[..., None] * ones        # (256,16,128)
    cNVx = (-v[..., 0])[..., None] * ones
    cNWvy = (-wv[..., 1])[..., None] * ones
    cIvy = ii - v[..., 1][..., None]

    allc = np.stack([cNWvx, cNVx, cNWvy, cIvy], axis=2)      # (256,16,4,128)
    allc = allc.reshape(N_CORES, G_CORE, 4, P).transpose(0, 3, 2, 1)
    coefs = np.ascontiguousarray(allc.reshape(N_CORES, P, 4 * G_CORE))

    thick = np.float32(np.asarray(thickness))
    r = thick / np.float32(2.0)
    invr2 = np.float32(1.0) / (r * r)

    # rhs rows per segment: [B]*128 ; A*j + C
    jj = np.arange(P, dtype=np.float32)
    r0 = np.broadcast_to(Bc[..., None], Bc.shape + (P,))      # (256,16,128)
    r1 = A[..., None] * jj + C[..., None]

    in_maps = []
    for core in range(N_CORES):
        allin = np.zeros((P, W_ALL2), dtype=np.float32)
        allin[:, 0:4 * G_CORE] = coefs[core]
        allin[:, X_OFF:X_OFF + P] = jj[None, :]
        allin[:, IR_OFF] = invr2
        allin[:, IR_OFF + 1] = np.float32(7.0)
        allin[0:1, LH_OFF:LH_OFF + P] = ii
        allin[1:2, LH_OFF:LH_OFF + P] = 1.0
        rh = np.zeros((2 * B_CORE, NSEG * P), dtype=np.float32)
        k0 = core * B_CORE
        for k in range(B_CORE):
            for s in range(NSEG):
                rh[2 * k, s * P:(s + 1) * P] = r0[k0 + k, s]
                rh[2 * k + 1, s * P:(s + 1) * P] = r1[k0 + k, s]
        in_maps.append({"allin": allin, "rhin": rh})
    return in_maps


def kernel(traj, thickness):
    if "nc" not in _cached:
        _cached["nc"] = _build_bass()
    in_maps = _host_coefs(traj, thickness)
    res = run_bass_kernel_spmd(_cached["nc"], in_maps, list(range(N_CORES)))
    return np.concatenate([res.results[c]["out"] for c in range(N_CORES)], axis=0)

